# revision 28
# baseline (speedup 1.0000x reference)
"""GroupSort (pairwise channel sort) Trainium2 Bass kernel.

out[:, 2k]   = min(x[:, 2k], x[:, 2k+1])
out[:, 2k+1] = max(x[:, 2k], x[:, 2k+1])

x: [32, 512, 56, 56] f32.  Batch-sharded across 8 NeuronCores (4 per core).
Per core the shard [4, 512, 56, 56] is viewed as [1024, 6272]: each row is
one (batch, channel-pair) - first 3136 cols = even channel's H*W pixels,
last 3136 = odd channel's.  Memory-bound: 25.7 MB in + 25.7 MB out per core.

Compute is replicated bit-exactly from the reference:
  z = relu(xe - xo); out_e = xe - z; out_o = xo + z
DVE does the three tensor_tensor ops (f32 1x mode), ACT does the relu.
Outputs overwrite the input tile, so each tile needs one full-row store
(25 KiB descriptors) instead of two half-row ones.

SDMA engine 15 sustains only ~22.4 GB/s vs ~26.3 for engines 0-14
(measured), and a DMA's descriptors are split over the largest queue
count <= 16 that divides the partition count (verified by probe: 128
partitions -> 16 queues x 8, 120 -> 15 queues x 8 on engines 0-14,
127 -> ONE engine).  Measured caveat: 15-way-split LOADS run ~2x slow
per descriptor (SBUF-write port crossing?), while 15-way STORES run at
line rate when the load queue is also active.  So all loads are
[128]-row tiles, and the rebalance is store-side only: the FIRST two
tiles store as [120 rows] (engines 0-14) + [8 rows] (engines 0-7).
Engine 15 ends up with 112 descriptors vs 130 for engines 0-7 -- every
engine finishes in ~125-130 us instead of engine 15 dragging the
uniform layout to ~145 us.  N_SPLIT=2 was tuned on hardware: 1 and 3
both regress to ~165 us.
"""

import os
import sys

import numpy as np

sys.path.insert(0, "/opt/trn_rl_repo")

import concourse.tile as tile
from concourse import bacc, mybir
from concourse.bass_utils import run_bass_kernel_spmd

def _install_trace_shim():
    """The image's antenv package lacks axon_hooks, which
    run_bass_kernel_spmd imports for trace=True. Install the same
    ctypes-based NTFF hook trn_boot would have registered, and keep
    profile artifacts local instead of uploading to a bucket."""
    try:
        import types as _types

        from concourse import bass_utils as _bu

        _bu.upload_artifacts = lambda tmpdir: tmpdir
        if "antenv.axon_hooks" not in sys.modules:
            from trn_agent_boot.trn_boot import _ntff_profile_via_ctypes

            _hook = _ntff_profile_via_ctypes("/opt/axon/libaxon_pjrt.so")
            _mod = _types.ModuleType("antenv.axon_hooks")
            _mod.get_axon_ntff_profile_hook = lambda: _hook
            _mod.set_axon_ntff_profile_hook = lambda h: None
            sys.modules["antenv.axon_hooks"] = _mod
    except Exception:
        pass


N_CORES = 8
B, C, H, W = 32, 512, 56, 56
HW = H * W  # 3136
B_PER = B // N_CORES  # 4
ROWS = B_PER * C // 2  # 1024 pair-rows per core
COLS = 2 * HW  # 6272
P = 128
N_TILES = ROWS // P  # 8

_cache = {}


def _build_nc():
    nc = bacc.Bacc(
        "TRN2",
        debug=False,
        num_devices=N_CORES,
        enable_partition_id=False,
        # We issue no SWDGE (gpsimd) DMAs, so the 16 KiB descriptor-ring
        # scratch can shrink -- frees SBUF for a 7th input buffer.
        dynamic_dma_scratch_size=2048,
    )
    x = nc.dram_tensor("x", [ROWS, COLS], mybir.dt.float32, kind="ExternalInput").ap()
    o = nc.dram_tensor(
        "out", [ROWS, COLS], mybir.dt.float32, kind="ExternalOutput"
    ).ap()

    relu = mybir.ActivationFunctionType.Relu
    # Tiles whose stores skip engine 15.  Placed FIRST: 15-way-split
    # stores only run at line rate while the load queue is also active
    # (measured); at the tail they run ~2x slow, so the tail stays [128].
    N_SPLIT = 2

    HHW = HW // 2  # column half

    with tile.TileContext(nc, num_cores=N_CORES) as tc:
        with (
            tc.tile_pool(name="inp", bufs=7) as inp,
            tc.tile_pool(name="zp", bufs=2) as zp,
        ):
            # Software-pipelined emission over half-column units: v(u+1)
            # on DVE and relu(u+1) on ACT are issued BEFORE oute/outo(u)
            # and store(u), so the ACT stream never blocks relu(u+1)
            # behind store(u)'s sem wait and the DVE conveyor never
            # stalls on ACT.  z is computed per column half so the z pool
            # is half-sized, leaving SBUF for the 7th input buffer.
            units = [(t, h) for t in range(N_TILES) for h in (0, 1)]
            its, zts = {}, {}

            def stage_front(u):
                t, h = units[u]
                if h == 0:
                    it = inp.tile([P, COLS], mybir.dt.float32)
                    if t == 0:
                        # First tile loads in 4 column pieces so v(0,0)
                        # only waits for the first half's xe+xo columns
                        # (~5 us earlier compute start).
                        for c0, c1 in (
                            (0, HHW),
                            (HW, HW + HHW),
                            (HHW, HW),
                            (HW + HHW, COLS),
                        ):
                            nc.sync.dma_start(
                                out=it[:, c0:c1], in_=x[0:P, c0:c1]
                            )
                    else:
                        nc.sync.dma_start(
                            out=it[:], in_=x[t * P : (t + 1) * P, :]
                        )
                    its[t] = it
                it = its[t]
                a, b = h * HHW, (h + 1) * HHW
                zt = zp.tile([P, HHW], mybir.dt.float32)
                nc.vector.tensor_sub(zt[:], it[:, a:b], it[:, HW + a : HW + b])
                nc.scalar.activation(zt[:], zt[:], relu)
                zts[u] = zt

            stage_front(0)
            for u in range(len(units)):
                if u + 1 < len(units):
                    stage_front(u + 1)
                t, h = units[u]
                it, zt = its[t], zts[u]
                a, b = h * HHW, (h + 1) * HHW
                nc.vector.tensor_sub(it[:, a:b], it[:, a:b], zt[:])
                nc.vector.tensor_add(
                    it[:, HW + a : HW + b], it[:, HW + a : HW + b], zt[:]
                )
                if h == 1:
                    # full-row stores (25 KiB contiguous per partition)
                    r = t * P
                    if t < N_SPLIT:
                        # split store: [120] -> eng 0-14, [8] -> eng 0-7
                        nc.scalar.dma_start(
                            out=o[r : r + 120, :], in_=it[0:120, :]
                        )
                        nc.scalar.dma_start(
                            out=o[r + 120 : r + P, :], in_=it[120:P, :]
                        )
                    else:
                        nc.scalar.dma_start(out=o[r : r + P, :], in_=it[:])
    nc.compile()
    return nc


def _get_nc():
    if "nc" not in _cache:
        _cache["nc"] = _build_nc()
    return _cache["nc"]


def kernel(
    x: np.ndarray,
    _trace: bool = False,
    _tmpdir: str | None = None,
    _trace_cores: list | None = None,
):
    assert x.shape == (B, C, H, W), x.shape
    x = np.ascontiguousarray(x, dtype=np.float32)
    shards = x.reshape(N_CORES, ROWS, COLS)
    in_maps = [{"x": shards[i]} for i in range(N_CORES)]

    nc = _get_nc()
    if _trace:
        _install_trace_shim()
        os.environ.pop("BASS_NEVER_TRACE", None)
    else:
        # run_bass_kernel_spmd also enables tracing when BASS_TRACE is set
        # in the environment; keep the grading path deterministic.
        os.environ["BASS_NEVER_TRACE"] = "1"
    res = run_bass_kernel_spmd(
        nc,
        in_maps,
        list(range(N_CORES)),
        trace=_trace,
        tmpdir=_tmpdir,
        trace_cores=_trace_cores,
    )
    out = np.empty((N_CORES, ROWS, COLS), dtype=np.float32)
    for i in range(N_CORES):
        out[i] = res.results[i]["out"]
    if _trace:
        kernel.last_exec_time_ns = res.exec_time_ns
        kernel.last_results = res
    return out.reshape(B, C, H, W)


if __name__ == "__main__":
    rng = np.random.default_rng(0)
    xt = rng.standard_normal((B, C, H, W), dtype=np.float32)
    yt = kernel(xt)
    xe, xo = xt[:, 0::2], xt[:, 1::2]
    z = np.maximum(xe - xo, 0)
    exp = np.empty_like(xt)
    exp[:, 0::2] = xe - z
    exp[:, 1::2] = xo + z
    err = np.abs(yt - exp).max()
    print("absmax err:", err)


# revision 29
# speedup vs baseline: 1.1312x; 1.1312x over previous
"""GroupSort (pairwise channel sort) Trainium2 Bass kernel.

out[:, 2k]   = min(x[:, 2k], x[:, 2k+1])
out[:, 2k+1] = max(x[:, 2k], x[:, 2k+1])

x: [32, 512, 56, 56] f32.  Batch-sharded across 8 NeuronCores (4 per core).
Per core the shard [4, 512, 56, 56] is viewed as [1024, 6272]: each row is
one (batch, channel-pair) - first 3136 cols = even channel's H*W pixels,
last 3136 = odd channel's.  Memory-bound: 25.7 MB in + 25.7 MB out per core.

Compute is replicated bit-exactly from the reference:
  z = relu(xe - xo); out_e = xe - z; out_o = xo + z
DVE does the three tensor_tensor ops (f32 1x mode), ACT does the relu.
Outputs overwrite the input tile, so each tile needs one full-row store
(25 KiB descriptors) instead of two half-row ones.

SDMA engine 15 sustains only ~22.4 GB/s vs ~26.3 for engines 0-14
(measured), and a DMA's descriptors are split over the largest queue
count <= 16 that divides the partition count (verified by probe: 128
partitions -> 16 queues x 8, 120 -> 15 queues x 8 on engines 0-14,
127 -> ONE engine).  Measured caveat: 15-way-split LOADS run ~2x slow
per descriptor (SBUF-write port crossing?), while 15-way STORES run at
line rate when the load queue is also active.  So all loads are
[128]-row tiles, and the rebalance is store-side only: the FIRST two
tiles store as [120 rows] (engines 0-14) + [8 rows] (engines 0-7).
Engine 15 ends up with 112 descriptors vs 130 for engines 0-7 -- every
engine finishes in ~125-130 us instead of engine 15 dragging the
uniform layout to ~145 us.  N_SPLIT=2 was tuned on hardware: 1 and 3
both regress to ~165 us.
"""

import os
import sys

import numpy as np

sys.path.insert(0, "/opt/trn_rl_repo")

import concourse.tile as tile
from concourse import bacc, mybir
from concourse.bass_utils import run_bass_kernel_spmd

def _install_trace_shim():
    """The image's antenv package lacks axon_hooks, which
    run_bass_kernel_spmd imports for trace=True. Install the same
    ctypes-based NTFF hook trn_boot would have registered, and keep
    profile artifacts local instead of uploading to a bucket."""
    try:
        import types as _types

        from concourse import bass_utils as _bu

        _bu.upload_artifacts = lambda tmpdir: tmpdir
        if "antenv.axon_hooks" not in sys.modules:
            from trn_agent_boot.trn_boot import _ntff_profile_via_ctypes

            _hook = _ntff_profile_via_ctypes("/opt/axon/libaxon_pjrt.so")
            _mod = _types.ModuleType("antenv.axon_hooks")
            _mod.get_axon_ntff_profile_hook = lambda: _hook
            _mod.set_axon_ntff_profile_hook = lambda h: None
            sys.modules["antenv.axon_hooks"] = _mod
    except Exception:
        pass


N_CORES = 8
B, C, H, W = 32, 512, 56, 56
HW = H * W  # 3136
B_PER = B // N_CORES  # 4
ROWS = B_PER * C // 2  # 1024 pair-rows per core
COLS = 2 * HW  # 6272
P = 128
N_TILES = ROWS // P  # 8

_cache = {}


def _build_nc():
    nc = bacc.Bacc(
        "TRN2",
        debug=False,
        num_devices=N_CORES,
        enable_partition_id=False,
        # We issue no SWDGE (gpsimd) DMAs, so the 16 KiB descriptor-ring
        # scratch can shrink -- frees SBUF for a 7th input buffer.
        dynamic_dma_scratch_size=2048,
    )
    x = nc.dram_tensor("x", [ROWS, COLS], mybir.dt.float32, kind="ExternalInput").ap()
    o = nc.dram_tensor(
        "out", [ROWS, COLS], mybir.dt.float32, kind="ExternalOutput"
    ).ap()

    relu = mybir.ActivationFunctionType.Relu
    # Tiles whose stores skip engine 15.  Placed FIRST: 15-way-split
    # stores only run at line rate while the load queue is also active
    # (measured); at the tail they run ~2x slow, so the tail stays [128].
    N_SPLIT = 2

    HHW = HW // 2  # column half

    with tile.TileContext(nc, num_cores=N_CORES) as tc:
        with (
            tc.tile_pool(name="inp", bufs=7) as inp,
            tc.tile_pool(name="zp", bufs=2) as zp,
        ):
            # Software-pipelined emission over half-column units: v(u+1)
            # on DVE and relu(u+1) on ACT are issued BEFORE oute/outo(u)
            # and store(u), so the ACT stream never blocks relu(u+1)
            # behind store(u)'s sem wait and the DVE conveyor never
            # stalls on ACT.  z is computed per column half so the z pool
            # is half-sized, leaving SBUF for the 7th input buffer.
            units = [(t, h) for t in range(N_TILES) for h in (0, 1)]
            its, zts = {}, {}

            def stage_front(u):
                t, h = units[u]
                if h == 0:
                    it = inp.tile([P, COLS], mybir.dt.float32)
                    nc.sync.dma_start(out=it[:], in_=x[t * P : (t + 1) * P, :])
                    its[t] = it
                it = its[t]
                a, b = h * HHW, (h + 1) * HHW
                zt = zp.tile([P, HHW], mybir.dt.float32)
                nc.vector.tensor_sub(zt[:], it[:, a:b], it[:, HW + a : HW + b])
                nc.scalar.activation(zt[:], zt[:], relu)
                zts[u] = zt

            stage_front(0)
            for u in range(len(units)):
                if u + 1 < len(units):
                    stage_front(u + 1)
                t, h = units[u]
                it, zt = its[t], zts[u]
                a, b = h * HHW, (h + 1) * HHW
                nc.vector.tensor_sub(it[:, a:b], it[:, a:b], zt[:])
                nc.vector.tensor_add(
                    it[:, HW + a : HW + b], it[:, HW + a : HW + b], zt[:]
                )
                if h == 1:
                    # full-row stores (25 KiB contiguous per partition)
                    r = t * P
                    if t < N_SPLIT:
                        # split store: [120] -> eng 0-14, [8] -> eng 0-7
                        nc.scalar.dma_start(
                            out=o[r : r + 120, :], in_=it[0:120, :]
                        )
                        nc.scalar.dma_start(
                            out=o[r + 120 : r + P, :], in_=it[120:P, :]
                        )
                    else:
                        nc.scalar.dma_start(out=o[r : r + P, :], in_=it[:])
    nc.compile()
    return nc


def _get_nc():
    if "nc" not in _cache:
        _cache["nc"] = _build_nc()
    return _cache["nc"]


def kernel(
    x: np.ndarray,
    _trace: bool = False,
    _tmpdir: str | None = None,
    _trace_cores: list | None = None,
):
    assert x.shape == (B, C, H, W), x.shape
    x = np.ascontiguousarray(x, dtype=np.float32)
    shards = x.reshape(N_CORES, ROWS, COLS)
    in_maps = [{"x": shards[i]} for i in range(N_CORES)]

    nc = _get_nc()
    if _trace:
        _install_trace_shim()
        os.environ.pop("BASS_NEVER_TRACE", None)
    else:
        # run_bass_kernel_spmd also enables tracing when BASS_TRACE is set
        # in the environment; keep the grading path deterministic.
        os.environ["BASS_NEVER_TRACE"] = "1"
    res = run_bass_kernel_spmd(
        nc,
        in_maps,
        list(range(N_CORES)),
        trace=_trace,
        tmpdir=_tmpdir,
        trace_cores=_trace_cores,
    )
    out = np.empty((N_CORES, ROWS, COLS), dtype=np.float32)
    for i in range(N_CORES):
        out[i] = res.results[i]["out"]
    if _trace:
        kernel.last_exec_time_ns = res.exec_time_ns
        kernel.last_results = res
    return out.reshape(B, C, H, W)


if __name__ == "__main__":
    rng = np.random.default_rng(0)
    xt = rng.standard_normal((B, C, H, W), dtype=np.float32)
    yt = kernel(xt)
    xe, xo = xt[:, 0::2], xt[:, 1::2]
    z = np.maximum(xe - xo, 0)
    exp = np.empty_like(xt)
    exp[:, 0::2] = xe - z
    exp[:, 1::2] = xo + z
    err = np.abs(yt - exp).max()
    print("absmax err:", err)
